# revision 1
# baseline (speedup 1.0000x reference)
"""Causal self-attention (B=4, T=2048, E=1024, H=16, D=64) on 8 TRN2 NeuronCores.

Sharding: data-parallel over batch (4) x tensor-parallel over heads (2 groups
of 8).  Core c handles batch b=c//2, head group g=c%2.

Per-core pipeline (all matmuls bf16 on TensorE, fp32 PSUM accumulation):
  A) qkv projection from pre-transposed x^T: q^T,k^T in [feat, tok] layout,
     v in natural [tok, feat] layout with a ones column per head (so the av
     matmul's 65th output row accumulates the softmax denominator Z).
  B) per head, per 1024-wide q window: scores^T = k^T_blk.T @ q^T into a
     2-bank PSUM tile -> one wide exp per k-block (ScalarE, scale=1/8, no
     max-subtraction: |scores|<4 for this data; causally-dead columns are
     trimmed, the diagonal 128x128 gets a triangular mask multiply on DVE)
     -> y^T[65, q] accumulation with v_aug -> normalize by 1/Z (DVE
     reciprocal + GpSimd partition_broadcast + DVE multiply).
     ScalarE's exp is the stage-B critical path, so the next pair's q/k
     projection matmuls are interleaved into the k-block loop as PE filler.
  C) output projection partials + const/2 (both pair cores add half, so the
     ReduceScatter sum restores the full constant) -> 4 chunked
     ReduceScatters over the neighbor pair, each DMA'd DRAM->DRAM straight
     to the output: chunk k reduces out-feat blocks {2k, 2k+1}; the pair's
     even core receives block 2k, the odd core 2k+1 (host reassembles).

Bias algebra: k bias is softmax-shift-invariant (dropped); v bias commutes
with the (row-stochastic) attention weights so it is folded with proj_b
into the output constant on the host; q bias is applied on-device.
"""

import sys

if "/opt/trn_rl_repo" not in sys.path:
    sys.path.insert(0, "/opt/trn_rl_repo")

import ml_dtypes
import numpy as np

import concourse.bass as bass
import concourse.mybir as mybir
import concourse.tile as tile
from concourse import bacc
from concourse.bass_utils import run_bass_kernel_spmd

B, T, E = 4, 2048, 1024
H, D = 16, 64
N_CORES = 8
F = 512          # local features per core (8 heads * 64)
HPC = 8          # heads per core
EC = E // 128    # 8 emb chunks
TC = T // 512    # 4 token chunks of 512
TB = T // 128    # 16 token blocks of 128
FB = F // 128    # 4 local feature blocks
OB = E // 128    # 8 output feature blocks
SCALE = 0.125    # 1/sqrt(D)

BF16 = mybir.dt.bfloat16
F32 = mybir.dt.float32
_nbf16 = ml_dtypes.bfloat16

_CACHED_NC = None


def build_nc(repeat=1, single_core=False):
    nc = bacc.Bacc("TRN2", target_bir_lowering=False, debug=False,
                   num_devices=1 if single_core else N_CORES)

    xT = nc.declare_dram_parameter("xT", [E, T], BF16, isOutput=False)
    wqT = nc.declare_dram_parameter("wqT", [E, F], BF16, isOutput=False)
    wkT = nc.declare_dram_parameter("wkT", [E, F], BF16, isOutput=False)
    wvT = nc.declare_dram_parameter("wvT", [E, F], BF16, isOutput=False)
    pwT = nc.declare_dram_parameter("pwT", [F, E], BF16, isOutput=False)
    bqd = nc.declare_dram_parameter("bq", [128, FB], F32, isOutput=False)
    cvd = nc.declare_dram_parameter("constv", [128, OB], F32, isOutput=False)
    out = nc.declare_dram_parameter("out", [F, T], F32, isOutput=True)

    AF = mybir.ActivationFunctionType
    ALU = mybir.AluOpType

    with tile.TileContext(nc) as tc:
        with (
            tc.tile_pool(name="persist", bufs=1) as pers,
            tc.tile_pool(name="work", bufs=6) as work,
            tc.tile_pool(name="evac", bufs=3) as evac,
            tc.tile_pool(name="psP", bufs=3, space="PSUM") as psP,
            tc.tile_pool(name="dram", bufs=1, space="DRAM") as dram,
        ):
            # ---- constants ----
            bq_t = pers.tile([128, FB], F32, tag="bq")
            cv_t = pers.tile([128, OB], F32, tag="cv")
            nc.sync.dma_start(bq_t[:], bqd[:])
            nc.sync.dma_start(cv_t[:], cvd[:])

            # upper-triangular (incl diag) ones [128, 128] bf16 for the
            # in-window diagonal block mask
            tri = pers.tile([128, 128], BF16, tag="tri")
            nc.gpsimd.memset(tri[:], 0.0)
            nc.gpsimd.affine_select(
                out=tri[:], in_=tri[:],
                compare_op=ALU.is_gt, fill=1.0,
                base=0, pattern=[[-1, 128]], channel_multiplier=1,
            )

            for _rep in range(repeat):
                # ---- persistent activations / weights ----
                xt = [pers.tile([128, T], BF16, tag=f"xT{ec}", name=f"xT{ec}") for ec in range(EC)]
                wq = [pers.tile([128, F], BF16, tag=f"wq{ec}", name=f"wq{ec}") for ec in range(EC)]
                wk = [pers.tile([128, F], BF16, tag=f"wk{ec}", name=f"wk{ec}") for ec in range(EC)]
                wv = [pers.tile([128, F], BF16, tag=f"wv{ec}", name=f"wv{ec}") for ec in range(EC)]
                pw = [pers.tile([128, E], BF16, tag=f"pw{fc}", name=f"pw{fc}") for fc in range(FB)]
                for ec in range(EC):
                    sl = slice(ec * 128, (ec + 1) * 128)
                    nc.sync.dma_start(xt[ec][:, 0:1024], xT[sl, 0:1024])
                    nc.sync.dma_start(wq[ec][:], wqT[sl, :])
                for ec in range(EC):
                    sl = slice(ec * 128, (ec + 1) * 128)
                    nc.sync.dma_start(wk[ec][:], wkT[sl, :])
                    nc.sync.dma_start(wv[ec][:], wvT[sl, :])
                for ec in range(EC):
                    sl = slice(ec * 128, (ec + 1) * 128)
                    nc.sync.dma_start(xt[ec][:, 1024:T], xT[sl, 1024:T])
                for fc in range(FB):
                    nc.sync.dma_start(pw[fc][:], pwT[fc * 128:(fc + 1) * 128, :])

                qT = [pers.tile([128, T], BF16, tag=f"qT{fb}", name=f"qT{fb}") for fb in range(FB)]
                kT = [pers.tile([128, T], BF16, tag=f"kT{fb}", name=f"kT{fb}") for fb in range(FB)]
                # v natural layout with per-head ones column: [vh(64) | 1] * 8
                va = [pers.tile([128, 520], BF16, tag=f"va{tb}", name=f"va{tb}") for tb in range(TB)]
                yT = [pers.tile([128, T], BF16, tag=f"yT{fb}", name=f"yT{fb}") for fb in range(FB)]

                # ---- stage A: qkv projections ([128,1024] psum windows) ----
                def qk_unit(fb, w2, which):
                    fsl = slice(fb * 128, (fb + 1) * 128)
                    wgt, dst, bias = ((wq, qT, True) if which == "q"
                                      else (wk, kT, False))
                    ps = psP.tile([128, 1024], F32, tag="big", name="psA")
                    for half in range(2):
                        tsl = slice(w2 * 1024 + half * 512,
                                    w2 * 1024 + (half + 1) * 512)
                        psl = slice(half * 512, (half + 1) * 512)
                        for ec in range(EC):
                            nc.tensor.matmul(
                                ps[:, psl], wgt[ec][:, fsl], xt[ec][:, tsl],
                                start=(ec == 0), stop=(ec == EC - 1))
                    wsl = slice(w2 * 1024, (w2 + 1) * 1024)
                    if bias:
                        nc.vector.tensor_scalar_add(dst[fb][:, wsl], ps[:],
                                                    bq_t[:, fb:fb + 1])
                    else:
                        nc.vector.tensor_copy(dst[fb][:, wsl], ps[:])

                def qk_proj(fb):
                    for w2 in range(T // 1024):
                        qk_unit(fb, w2, "q")
                        qk_unit(fb, w2, "k")

                def v_proj(tb):
                    bsl = slice(tb * 128, (tb + 1) * 128)
                    ps = psP.tile([128, 512], F32, tag="big", name="psV")
                    for ec in range(EC):
                        nc.tensor.matmul(ps[:], xt[ec][:, bsl], wv[ec][:],
                                         start=(ec == 0), stop=(ec == EC - 1))
                    nc.gpsimd.memset(va[tb][:], 1.0)
                    # one strided copy: [128, 8x64] -> cols {65h..65h+63}
                    nc.vector.tensor_copy(
                        va[tb].rearrange("p (h c) -> p h c", h=HPC)[:, :, 0:64],
                        ps[:].rearrange("p (h c) -> p h c", h=HPC))


                # ---- stage B: attention, 1024-wide q windows; PE filler
                # work (next pair's q/k projection) is injected between kb
                # blocks so the PE keeps producing while ACT drains exps ----
                fillers = []

                def normalize(h, qc, psy_t):
                    fb, po = h // 2, (h % 2) * 64
                    qsl = slice(qc * 512, (qc + 1) * 512)
                    # yT = psy[0:64] * (1/Z): row-broadcast 1/Z on Pool
                    rz = evac.tile([1, 512], BF16, tag="rz")
                    with nc.allow_low_precision(
                            reason="1/Z in bf16; Z is O(1e2), "
                            "0.4% relative is within budget"):
                        nc.vector.reciprocal(rz[:], psy_t[64:65, :])
                    zb = evac.tile([64, 512], BF16, tag="zb")
                    nc.gpsimd.partition_broadcast(zb[:], rz[:])
                    nc.vector.tensor_mul(yT[fb][po:po + 64, qsl],
                                         psy_t[0:64, :], zb[:])

                def attn_win(h, w, jit_v=False):
                    fb, po = h // 2, (h % 2) * 64
                    qh = qT[fb][po:po + 64, :]
                    kh = kT[fb][po:po + 64, :]
                    if True:
                        psy = {}
                        for qc in (2 * w, 2 * w + 1):
                            psy[qc] = psP.tile([65, 512], F32, tag="psy",
                                               bufs=2, name="psy")
                        for kb in range(8 * w + 8):
                            if jit_v and kb + 1 < TB // 2:
                                # first window: emit v blocks just ahead of
                                # their av consumers instead of all upfront
                                v_proj(kb + 1)
                            if kb == 8 * w + 4:
                                # even-qc psum complete: normalize now so its
                                # bank frees mid-window
                                normalize(h, 2 * w, psy[2 * w])
                            if fillers:
                                fillers.pop(0)()
                            j = kb - 8 * w
                            off = max(j, 0) * 128
                            pss = psP.tile([128, 1024], F32, tag="big",
                                           name="pss")
                            ksl = slice(kb * 128, (kb + 1) * 128)
                            for half in range(2):
                                lo = max(off, half * 512)
                                hi = (half + 1) * 512
                                if lo >= hi:
                                    continue
                                nc.tensor.matmul(
                                    pss[:, lo:hi], kh[:, ksl],
                                    qh[:, w * 1024 + lo:w * 1024 + hi],
                                    start=True, stop=True)
                            at = work.tile([128, 1024], BF16, tag="attT")
                            nc.scalar.activation(at[:, off:1024], pss[:, off:1024],
                                                 AF.Exp, scale=SCALE)
                            if j >= 0:
                                nc.vector.tensor_mul(at[:, off:off + 128],
                                                     at[:, off:off + 128], tri[:])
                            for halfq in range(2):
                                qc = 2 * w + halfq
                                if kb > 4 * qc + 3:
                                    continue
                                lo = max(off, halfq * 512)
                                hi = (halfq + 1) * 512
                                nc.tensor.matmul(
                                    psy[qc][:, lo - halfq * 512:hi - halfq * 512],
                                    va[kb][:, h * 65:h * 65 + 65], at[:, lo:hi],
                                    start=(kb == 0), stop=(kb == 4 * qc + 3))
                        normalize(h, 2 * w + 1, psy[2 * w + 1])

                qk_unit(0, 0, "q")
                qk_unit(0, 0, "k")
                v_proj(0)
                fillers.append(lambda: qk_unit(0, 1, "q"))
                fillers.append(lambda: qk_unit(0, 1, "k"))
                vfill = [(lambda tb=tb: v_proj(tb)) for tb in range(8, TB)]
                fillers.extend(vfill)
                fillers.extend(
                    (lambda f=f, w2=w2, wh=wh: qk_unit(f, w2, wh))
                    for f in range(1, FB)
                    for w2 in range(T // 1024) for wh in ("q", "k"))
                for pair in range(FB):
                    attn_win(2 * pair, 0, jit_v=(pair == 0))
                    attn_win(2 * pair + 1, 0)
                    if pair == 0:
                        # emission-order dependency: va[8..15] and qk0's
                        # second window must be emitted before w=1 consumes
                        while any(f in vfill for f in fillers):
                            fillers.pop(0)()
                    attn_win(2 * pair, 1)
                    attn_win(2 * pair + 1, 1)
                while fillers:
                    fillers.pop(0)()

                # ---- stage C: projection + chunked ReduceScatter ----
                # chunk k reduces out-feat blocks {2k, 2k+1}; the pair's even
                # core receives block 2k, the odd core block 2k+1 (host
                # reassembles).  Chunking overlaps RS/final with later proj.
                for ck in range(OB // 2):
                    yTp = dram.tile([256, T], F32, tag="yTp", name=f"yTp{ck}")
                    yTr = dram.tile([128, T], F32, tag="yTr", name=f"yTr{ck}")
                    for obh in range(2):
                        ob = 2 * ck + obh
                        osl = slice(ob * 128, (ob + 1) * 128)
                        for w2 in range(T // 1024):
                            ps = psP.tile([128, 1024], F32, tag="big",
                                          name="psC")
                            for half in range(2):
                                tsl = slice(w2 * 1024 + half * 512,
                                            w2 * 1024 + (half + 1) * 512)
                                psl = slice(half * 512, (half + 1) * 512)
                                for fc in range(FB):
                                    nc.tensor.matmul(ps[:, psl], pw[fc][:, osl],
                                                     yT[fc][:, tsl],
                                                     start=(fc == 0),
                                                     stop=(fc == FB - 1))
                            st = evac.tile([128, 1024], F32, tag="pjevac")
                            # + const/2 here: both pair cores add half, the
                            # ReduceScatter sum restores the full constant
                            nc.vector.tensor_scalar_add(st[:], ps[:],
                                                        cv_t[:, ob:ob + 1])
                            nc.sync.dma_start(
                                yTp[obh * 128:(obh + 1) * 128,
                                    w2 * 1024:(w2 + 1) * 1024], st[:])
                    if single_core:
                        # timeline-sim stand-in for the pair ReduceScatter
                        nc.sync.dma_start(yTr[:], yTp[0:128, :])
                    else:
                        nc.gpsimd.collective_compute(
                            "ReduceScatter",
                            ALU.add,
                            replica_groups=[[0, 1], [2, 3], [4, 5], [6, 7]],
                            ins=[yTp.opt()],
                            outs=[yTr.opt()],
                        )
                    nc.sync.dma_start(out[ck * 128:(ck + 1) * 128, :], yTr[:])

    nc.compile()
    return nc


def _get_nc():
    global _CACHED_NC
    if _CACHED_NC is None:
        _CACHED_NC = build_nc()
    return _CACHED_NC


def make_in_maps(x, qkv_w, qkv_b, proj_w, proj_b):
    x = np.asarray(x, np.float32)
    qkv_w = np.asarray(qkv_w, np.float32)
    qkv_b = np.asarray(qkv_b, np.float32)
    proj_w = np.asarray(proj_w, np.float32)
    proj_b = np.asarray(proj_b, np.float32)

    const = proj_b + proj_w @ qkv_b[2 * E:3 * E]  # v-bias folded through proj
    in_maps = []
    for c in range(N_CORES):
        b, g = c // 2, c % 2
        gsl = slice(g * F, (g + 1) * F)
        xTb = np.ascontiguousarray(x[b].T).astype(_nbf16)
        m = {
            "xT": xTb,
            "wqT": np.ascontiguousarray(qkv_w[gsl].T).astype(_nbf16),
            "wkT": np.ascontiguousarray(qkv_w[E + g * F:E + (g + 1) * F].T
                                        ).astype(_nbf16),
            "wvT": np.ascontiguousarray(qkv_w[2 * E + g * F:2 * E + (g + 1) * F].T
                                        ).astype(_nbf16),
            "pwT": np.ascontiguousarray(proj_w[:, gsl].T).astype(_nbf16),
            "bq": np.ascontiguousarray(qkv_b[gsl].reshape(FB, 128).T
                                       ).astype(np.float32),
            # const/2 is added pre-ReduceScatter by both pair cores;
            # col ob = const[ob block] / 2
            "constv": np.ascontiguousarray(
                const.reshape(OB, 128).T / 2.0).astype(np.float32),
        }
        in_maps.append(m)
    return in_maps


def assemble_output(results):
    y = np.empty((B, T, E), np.float32)
    for c in range(N_CORES):
        b, g = c // 2, c % 2
        o = results[c]["out"]  # [512, T]: row block k = out-feat block 2k+g
        for k in range(FB):
            blk = 2 * k + g
            y[b][:, blk * 128:(blk + 1) * 128] = o[k * 128:(k + 1) * 128].T
    return y


def kernel(**inputs):
    nc = _get_nc()
    in_maps = make_in_maps(**inputs)
    res = run_bass_kernel_spmd(nc, in_maps, list(range(N_CORES)))
    return assemble_output(res.results)



# revision 14
# speedup vs baseline: 1.1809x; 1.1809x over previous
"""Causal self-attention (B=4, T=2048, E=1024, H=16, D=64) on 8 TRN2 NeuronCores.

Sharding: data-parallel over batch (4) x tensor-parallel over heads (2 groups
of 8).  Core c handles batch b=c//2, head group g=c%2.

Per-core pipeline (fp8 DoubleRow matmuls where accuracy allows, fp32 PSUM):
  A) qkv projections in fp8e4 DoubleRow (0.5 cyc/row, 256-deep contraction
     pairs): weights pre-scaled by 64 on the host (e4m3 can't represent the
     0.02-sigma weights) and split hi/residual, x split hi/residual; the
     3-term compensated product x1*W1 + x1*Wr + xr*W1 restores near-bf16
     accuracy at 6/8 the bf16 PE cost.  The 1/64 is folded into the DVE
     evacuation ops.  q (+bias) and k are written as fp8e4 [feat, tok] tiles
     for stage B; v in natural [tok, feat] bf16 layout with a ones column per
     head (av's 65th output row accumulates the softmax denominator Z).
     Inputs arrive as few large DMAs spread over 4 issue queues (SP/ACT/DVE/
     Pool) so sequencer+HWDGE serialization doesn't gate the first matmul.
  B) per head, per 1024-wide q window: scoresT = k_blk.T @ q via fp8
     DoubleRow with BOTH pair slots broadcast to the same data (stride-0
     second slot), which computes 2*(k.T q) at 0.5 cyc/row -- the doubling is
     folded into the exp scale (1/16).  Causally-dead columns are trimmed;
     the diagonal 128x128 block gets -240*strict-lower-mask accumulated via
     an extra fp8 DR matmul in the same PSUM group (exp then yields exact
     zeros).  One wide exp per k-block (ScalarE, scale=1/16, no
     max-subtraction: |scores|<4 for this data) -> y^T[65, q] accumulation
     with bf16 va (fp8 would breach the error budget) -> normalize by 1/Z
     (DVE reciprocal + GpSimd partition_broadcast + DVE multiply).
     ScalarE's exp is the stage-B critical path: projection filler matmuls
     are deadline-paced across all k-block slots to keep the PE fed without
     starving it late, and the odd-half psy accumulation trails two k-blocks
     behind so the previous window's normalize can free its PSUM bank
     without stalling this window's exp stream.
  C) output projection partials (bf16) + const/2 (both pair cores add half,
     so the ReduceScatter sum restores the full constant) -> 4 chunked
     ReduceScatters over the neighbor pair writing straight into the output
     DRAM tensor: chunk k reduces out-feat blocks {2k, 2k+1}; the pair's
     even core receives block 2k, the odd core 2k+1 (host reassembles).

Bias algebra: k bias is softmax-shift-invariant (dropped); v bias commutes
with the (row-stochastic) attention weights so it is folded with proj_b
into the output constant on the host; q bias is applied on-device.
"""

import sys

if "/opt/trn_rl_repo" not in sys.path:
    sys.path.insert(0, "/opt/trn_rl_repo")

import ml_dtypes
import numpy as np

import concourse.bass as bass
import concourse.mybir as mybir
import concourse.tile as tile
from concourse import bacc
from concourse.bass_utils import run_bass_kernel_spmd

B, T, E = 4, 2048, 1024
H, D = 16, 64
N_CORES = 8
F = 512          # local features per core (8 heads * 64)
HPC = 8          # heads per core
EC = E // 256    # 4 emb pair-chunks of 256 (fp8 DoubleRow)
TC = T // 512    # 4 token chunks of 512
TB = T // 128    # 16 token blocks of 128
FB = F // 128    # 4 local feature blocks
OB = E // 128    # 8 output feature blocks
SCALE = 0.125    # 1/sqrt(D)
WS = 64.0        # host-side weight prescale for fp8 range
INV_WS = 1.0 / WS

BF16 = mybir.dt.bfloat16
F32 = mybir.dt.float32
F8 = mybir.dt.float8e4
_nbf16 = ml_dtypes.bfloat16
_ne4m3 = ml_dtypes.float8_e4m3
DRMODE = mybir.MatmulPerfMode.DoubleRow

_CACHED_NC = None


def build_nc(repeat=1, single_core=False):
    nc = bacc.Bacc("TRN2", target_bir_lowering=False, debug=False,
                   num_devices=1 if single_core else N_CORES)

    # fp8 pair layouts, partition-major so each tensor is ONE dma:
    #   x1/xr [128, (c 4, i 2, t T)]  elem (p, c, i, t) = x[256c+128i+p, t]
    #   w*    [128, (c 4, i 2, hr 2, f F)]  hr: 0=hi, 1=residual
    x1d = nc.declare_dram_parameter("x1", [128, EC * 2 * T], F8, isOutput=False)
    xrd = nc.declare_dram_parameter("xr", [128, EC * 2 * T], F8, isOutput=False)
    wd = {nm: nc.declare_dram_parameter(f"w{nm}", [128, EC * 2 * 2 * F], F8,
                                        isOutput=False)
          for nm in ("q", "k", "v")}
    pwd = nc.declare_dram_parameter("pwT", [128, FB * E], BF16, isOutput=False)
    bqd = nc.declare_dram_parameter("bq", [128, FB], F32, isOutput=False)
    cvd = nc.declare_dram_parameter("constv", [128, OB], F32, isOutput=False)
    mkd = nc.declare_dram_parameter("maskp", [128, 256], F8, isOutput=False)
    # bf16 output: halves the stage-C DMA/ReduceScatter bytes; the host
    # converts back (bf16 rounding of the final values is ~0.1% rms)
    out = nc.declare_dram_parameter("out", [F, T], BF16, isOutput=True)

    AF = mybir.ActivationFunctionType
    ALU = mybir.AluOpType

    def bc2(ap, p, n):
        # add a stride-0 pair dim: [p, n] -> [p, 2, n]
        return ap.unsqueeze(1).broadcast_to([p, 2, n])

    with tile.TileContext(nc) as tc:
        with (
            tc.tile_pool(name="persist", bufs=1) as pers,
            tc.tile_pool(name="work", bufs=8) as work,
            tc.tile_pool(name="evac", bufs=3) as evac,
            tc.tile_pool(name="psP", bufs=3, space="PSUM") as psP,
            tc.tile_pool(name="dram", bufs=1, space="DRAM") as dram,
        ):
            # ---- constants (issued below, after the critical x/w loads) ----
            bq_t = pers.tile([128, FB], F32, tag="bq")
            cv_t = pers.tile([128, OB], F32, tag="cv")
            mk_t = pers.tile([128, 256], F8, tag="mask")

            for _rep in range(repeat):
                # ---- persistent activations / weights (fp8 pair layout) ----
                x1 = pers.tile([128, EC * 2 * T], F8, tag="x1", name="x1")
                xr = pers.tile([128, EC * 2 * T], F8, tag="xr", name="xr")
                wt = {nm: pers.tile([128, EC * 2 * 2 * F], F8, tag=f"w{nm}",
                                    name=f"w{nm}") for nm in ("q", "k", "v")}
                pw = pers.tile([128, FB * E], BF16, tag="pw", name="pw")

                def xview(t):
                    return t[:].rearrange("p (c two t) -> p c two t",
                                          c=EC, two=2)

                def wview(t):
                    return t[:].rearrange("p (c two hr f) -> p c two hr f",
                                          c=EC, two=2, hr=2)

                # few big DMAs; transfers serialize on the DMA fabric
                # (~0.385ns per partition-byte), so strictly order them by
                # first use: the first qk unit needs x halves 0 + wq + wk
                # (24KB/partition), everything else comes after.
                nc.sync.dma_start(
                    xview(x1)[:, :, :, 0:1024],
                    x1d[:].rearrange("p (c two t) -> p c two t",
                                     c=EC, two=2)[:, :, :, 0:1024])
                nc.scalar.dma_start(wt["q"][:], wd["q"][:])
                nc.gpsimd.dma_start(wt["k"][:], wd["k"][:])
                nc.sync.dma_start(bq_t[:], bqd[:])
                nc.sync.dma_start(mk_t[:], mkd[:])
                nc.gpsimd.dma_start(
                    xview(xr)[:, :, :, 0:1024],
                    xrd[:].rearrange("p (c two t) -> p c two t",
                                     c=EC, two=2)[:, :, :, 0:1024])
                nc.sync.dma_start(wt["v"][:], wd["v"][:])
                nc.scalar.dma_start(
                    xview(x1)[:, :, :, 1024:T],
                    x1d[:].rearrange("p (c two t) -> p c two t",
                                     c=EC, two=2)[:, :, :, 1024:T])
                nc.gpsimd.dma_start(
                    xview(xr)[:, :, :, 1024:T],
                    xrd[:].rearrange("p (c two t) -> p c two t",
                                     c=EC, two=2)[:, :, :, 1024:T])
                nc.scalar.dma_start(cv_t[:], cvd[:])
                nc.sync.dma_start(pw[:], pwd[:])

                qf8 = [pers.tile([128, T], F8, tag=f"qf{fb}", name=f"qf{fb}")
                       for fb in range(FB)]
                kf8 = [pers.tile([128, T], F8, tag=f"kf{fb}", name=f"kf{fb}")
                       for fb in range(FB)]
                # v natural layout with per-head ones column: [vh(64) | 1] * 8
                va = [pers.tile([128, 520], BF16, tag=f"va{tb}", name=f"va{tb}")
                      for tb in range(TB)]
                yT = [pers.tile([128, T], BF16, tag=f"yT{fb}", name=f"yT{fb}")
                      for fb in range(FB)]

                # ---- stage A: qkv projections, fp8 DR 3-term compensated.
                # Emitted in 512-token half-units so filler pacing can
                # spread the PE work finely between attention k-blocks. ----
                def qk_half(fb, w2, which, half):
                    fsl = slice(fb * 128, (fb + 1) * 128)
                    wkey, dst, bias = (("q", qf8, True) if which == "q"
                                       else ("k", kf8, False))
                    wv_ = wview(wt[wkey])
                    ps = psP.tile([128, 512], F32, tag="big", name="psA")
                    tsl = slice(w2 * 1024 + half * 512,
                                w2 * 1024 + (half + 1) * 512)
                    n = 0
                    for xv_, hr in ((x1, 0), (x1, 1), (xr, 0)):
                        for c in range(EC):
                            nc.tensor.matmul(
                                ps[:], wv_[:, c, :, hr, fsl],
                                xview(xv_)[:, c, :, tsl],
                                start=(n == 0), stop=(n == 11),
                                perf_mode=DRMODE)
                            n += 1
                    if bias:
                        nc.vector.tensor_scalar(dst[fb][:, tsl], ps[:],
                                                INV_WS, bq_t[:, fb:fb + 1],
                                                ALU.mult, ALU.add)
                    else:
                        nc.vector.tensor_scalar_mul(dst[fb][:, tsl], ps[:],
                                                    INV_WS)

                def qk_unit(fb, w2, which):
                    qk_half(fb, w2, which, 0)
                    qk_half(fb, w2, which, 1)

                def v_proj(tb):
                    bsl = slice(tb * 128, (tb + 1) * 128)
                    wv_ = wview(wt["v"])
                    ps = psP.tile([128, 512], F32, tag="big", name="psV")
                    n = 0
                    for xv_, hr in ((x1, 0), (x1, 1), (xr, 0)):
                        for c in range(EC):
                            nc.tensor.matmul(
                                ps[:], xview(xv_)[:, c, :, bsl],
                                wv_[:, c, :, hr, :],
                                start=(n == 0), stop=(n == 11),
                                perf_mode=DRMODE)
                            n += 1
                    nc.gpsimd.memset(va[tb][:], 1.0)
                    # one strided copy: [128, 8x64] -> cols {65h..65h+63}
                    nc.vector.tensor_scalar_mul(
                        va[tb].rearrange("p (h c) -> p h c", h=HPC)[:, :, 0:64],
                        ps[:].rearrange("p (h c) -> p h c", h=HPC), INV_WS)

                # ---- stage B: attention, 1024-wide q windows.  Filler units
                # (projections for later heads) are assigned static emission
                # slots: latest-possible by consumer deadline, then spread
                # backward with a minimum spacing so no window-start gets a
                # multi-unit PE burst that would stall the exp stream ----
                #
                # window order and start slots: pair p occupies
                # [48p, 48p+48): win(2p,0)=8, win(2p+1,0)=8, win(2p,1)=16,
                # win(2p+1,1)=16 slots.
                slot_fillers = {}  # slot -> [fn]
                state = {"slot": 0}

                def pace():
                    s = state["slot"]
                    state["slot"] += 1
                    for fn in slot_fillers.pop(s, ()):
                        fn()

                def normalize(h, qc, psy_t):
                    fb, po = h // 2, (h % 2) * 64
                    qsl = slice(qc * 512, (qc + 1) * 512)
                    # yT = psy[0:64] * (1/Z): row-broadcast 1/Z on Pool
                    rz = evac.tile([1, 512], BF16, tag="rz")
                    with nc.allow_low_precision(
                            reason="1/Z in bf16; Z is O(1e2), "
                            "0.4% relative is within budget"):
                        nc.vector.reciprocal(rz[:], psy_t[64:65, :])
                    zb = evac.tile([64, 512], BF16, tag="zb")
                    nc.gpsimd.partition_broadcast(zb[:], rz[:])
                    nc.vector.tensor_mul(yT[fb][po:po + 64, qsl],
                                         psy_t[0:64, :], zb[:])

                def attn_win(h, w, jit_v=False):
                    fb, po = h // 2, (h % 2) * 64
                    qh = qf8[fb][po:po + 64, :]
                    kh = kf8[fb][po:po + 64, :]
                    psy = {}
                    qe, qo = 2 * w, 2 * w + 1
                    psy[qe] = psP.tile([65, 512], F32, tag="psy",
                                       bufs=2, name="psy")
                    # odd-half av trails 2 k-blocks so the previous window's
                    # normalize can free this psum bank without stalling exp
                    odd_delay = []

                    def odd_av(kb, at):
                        if kb > 4 * qo + 3:
                            return
                        lo = max(max(kb - 8 * w, 0) * 128, 512)
                        nc.tensor.matmul(
                            psy[qo][:, lo - 512:1024 - 512],
                            va[kb][:, h * 65:h * 65 + 65], at[:, lo:1024],
                            start=(kb == 0), stop=(kb == 4 * qo + 3))

                    for kb in range(8 * w + 8):
                        if jit_v and kb + 1 < TB // 2:
                            # first window: emit v blocks just ahead of
                            # their av consumers instead of all upfront
                            v_proj(kb + 1)
                        if kb == 8 * w + 4:
                            # even-qc psum complete: normalize now so its
                            # bank frees mid-window
                            normalize(h, qe, psy[qe])
                        pace()
                        j = kb - 8 * w
                        off = max(j, 0) * 128
                        pss = psP.tile([128, 1024], F32, tag="big",
                                       name="pss")
                        ksl = slice(kb * 128, (kb + 1) * 128)
                        lhsT = bc2(kh[:, ksl], 64, 128)
                        for half in range(2):
                            lo = max(off, half * 512)
                            hi = (half + 1) * 512
                            if lo >= hi:
                                continue
                            if j >= 0 and lo == off and off < hi:
                                # diag block first, with the causal mask
                                # accumulated into the same PSUM group
                                nc.tensor.matmul(
                                    pss[:, off:off + 128], lhsT,
                                    bc2(qh[:, w * 1024 + off:
                                           w * 1024 + off + 128], 64, 128),
                                    start=True, stop=False,
                                    perf_mode=DRMODE)
                                nc.tensor.matmul(
                                    pss[:, off:off + 128],
                                    bc2(mk_t[:, 0:128], 128, 128),
                                    bc2(mk_t[:, 128:256], 128, 128),
                                    start=False, stop=True,
                                    perf_mode=DRMODE)
                                if off + 128 < hi:
                                    nc.tensor.matmul(
                                        pss[:, off + 128:hi], lhsT,
                                        bc2(qh[:, w * 1024 + off + 128:
                                               w * 1024 + hi],
                                            64, hi - off - 128),
                                        start=True, stop=True,
                                        perf_mode=DRMODE)
                            else:
                                nc.tensor.matmul(
                                    pss[:, lo:hi], lhsT,
                                    bc2(qh[:, w * 1024 + lo:
                                           w * 1024 + hi], 64, hi - lo),
                                    start=True, stop=True,
                                    perf_mode=DRMODE)
                        at = work.tile([128, 1024], BF16, tag="attT")
                        # scoresT psum = 2*s (doubled pair slots), so the
                        # exp scale is SCALE/2
                        nc.scalar.activation(at[:, off:1024],
                                             pss[:, off:1024],
                                             AF.Exp, scale=SCALE / 2)
                        # even-half av immediately
                        if kb <= 4 * qe + 3:
                            lo = off
                            hi = 512
                            if lo < hi:
                                nc.tensor.matmul(
                                    psy[qe][:, lo:hi],
                                    va[kb][:, h * 65:h * 65 + 65],
                                    at[:, lo:hi],
                                    start=(kb == 0), stop=(kb == 4 * qe + 3))
                        # odd-half av with a 2-slot lag
                        odd_delay.append((kb, at))
                        if kb == 1:
                            psy[qo] = psP.tile([65, 512], F32, tag="psy",
                                               bufs=2, name="psy")
                        if len(odd_delay) > 2:
                            okb, oat = odd_delay.pop(0)
                            odd_av(okb, oat)
                    for okb, oat in odd_delay:
                        odd_av(okb, oat)
                    normalize(h, qo, psy[qo])

                qk_unit(0, 0, "q")
                qk_unit(0, 0, "k")
                v_proj(0)
                # (deadline_slot, spacing, fn) for every deferred unit
                units = []
                for wh in ("q", "k"):
                    for half in range(2):
                        units.append((13, 3, lambda wh=wh, half=half:
                                      qk_half(0, 1, wh, half)))
                units.extend((21 + i, 1, (lambda tb=tb: v_proj(tb)))
                             for i, tb in enumerate(range(8, TB)))
                for f in range(1, FB):
                    s0 = 48 * f
                    for dl in (s0 - 3, s0 + 13):
                        w2 = 0 if dl == s0 - 3 else 1
                        for wh in ("q", "k"):
                            for half in range(2):
                                units.append((dl, 3,
                                              lambda f=f, w2=w2, wh=wh,
                                              half=half:
                                              qk_half(f, w2, wh, half)))
                # latest-possible by deadline, spread backward with spacing
                units.sort(key=lambda u: u[0])
                allowed = 191
                for dl, gap, fn in reversed(units):
                    s = max(0, min(dl, allowed))
                    slot_fillers.setdefault(s, []).insert(0, fn)
                    allowed = s - gap
                for pair in range(FB):
                    attn_win(2 * pair, 0, jit_v=(pair == 0))
                    attn_win(2 * pair + 1, 0)
                    attn_win(2 * pair, 1)
                    attn_win(2 * pair + 1, 1)
                assert not slot_fillers, slot_fillers.keys()

                # ---- stage C: projection + chunked ReduceScatter ----
                # chunk ob reduces out-feat block ob (bf16): the pair's even
                # core receives its first 64 features, the odd core the last
                # 64 (host reassembles).  Fine chunks overlap each RS with
                # the next block's projection; the RS writes straight into
                # the output DRAM tensor.
                for ob in range(OB):
                    yTp = dram.tile([128, T], BF16, tag="yTp", name=f"yTp{ob}")
                    yTr = dram.tile([64, T], BF16, tag="yTr", name=f"yTr{ob}")
                    osl_out = slice(ob * 64, (ob + 1) * 64)
                    for w2 in range(T // 1024):
                        ps = psP.tile([128, 1024], F32, tag="big",
                                      name="psC")
                        for half in range(2):
                            tsl = slice(w2 * 1024 + half * 512,
                                        w2 * 1024 + (half + 1) * 512)
                            psl = slice(half * 512, (half + 1) * 512)
                            for fc in range(FB):
                                nc.tensor.matmul(
                                    ps[:, psl],
                                    pw[:, fc * E + ob * 128:
                                       fc * E + (ob + 1) * 128],
                                    yT[fc][:, tsl],
                                    start=(fc == 0),
                                    stop=(fc == FB - 1))
                        st = evac.tile([128, 1024], BF16, tag="pjevac")
                        # + const/2 here: both pair cores add half, the
                        # ReduceScatter sum restores the full constant
                        nc.vector.tensor_scalar_add(st[:], ps[:],
                                                    cv_t[:, ob:ob + 1])
                        nc.sync.dma_start(
                            yTp[:, w2 * 1024:(w2 + 1) * 1024], st[:])
                    if single_core:
                        # timeline-sim stand-in for the pair ReduceScatter
                        nc.sync.dma_start(yTr[:], yTp[0:64, :])
                    else:
                        nc.gpsimd.collective_compute(
                            "ReduceScatter",
                            ALU.add,
                            replica_groups=[[0, 1], [2, 3], [4, 5], [6, 7]],
                            ins=[yTp.opt()],
                            outs=[yTr.opt()],
                        )
                    nc.sync.dma_start(out[osl_out, :], yTr[:])

    nc.compile()
    return nc


def _get_nc():
    global _CACHED_NC
    if _CACHED_NC is None:
        _CACHED_NC = build_nc()
    return _CACHED_NC


def _pairs_x(a):
    """[E, T] -> [128, (c, i, t)]: col c*2T + i*T + t = a[256c+128i+p, t]."""
    Edim, N = a.shape
    return np.ascontiguousarray(
        a.reshape(EC, 2, 128, N).transpose(2, 0, 1, 3)).reshape(128, -1)


def _pairs_w(hi, res):
    """two [E, F] -> [128, (c, i, hr, f)]."""
    h = hi.reshape(EC, 2, 128, F)
    r = res.reshape(EC, 2, 128, F)
    st = np.stack([h, r], axis=3)  # [c, i, 128, hr, F]
    return np.ascontiguousarray(st.transpose(2, 0, 1, 3, 4)).reshape(128, -1)


def _hi_res(a):
    hi = a.astype(_ne4m3)
    res = (a - hi.astype(np.float32)).astype(_ne4m3)
    return hi, res


def make_in_maps(x, qkv_w, qkv_b, proj_w, proj_b):
    x = np.asarray(x, np.float32)
    qkv_w = np.asarray(qkv_w, np.float32)
    qkv_b = np.asarray(qkv_b, np.float32)
    proj_w = np.asarray(proj_w, np.float32)
    proj_b = np.asarray(proj_b, np.float32)

    const = proj_b + proj_w @ qkv_b[2 * E:3 * E]  # v-bias folded through proj

    # causal mask blocks: [I | -240*strict_lower] fp8
    I128 = np.eye(128, dtype=np.float32)
    L128 = -240.0 * np.tril(np.ones((128, 128), np.float32), -1)
    maskp = np.concatenate([I128, L128], axis=1).astype(_ne4m3)

    # per-batch x hi/residual pair tiles (shared by the two g-groups)
    xsplit = []
    for b in range(B):
        xT = np.ascontiguousarray(x[b].T)  # [E, T]
        hi, res = _hi_res(xT)
        xsplit.append((_pairs_x(hi.astype(np.float32)).astype(_ne4m3),
                       _pairs_x(res.astype(np.float32)).astype(_ne4m3)))

    # per-group weight hi/residual pair tiles
    wsplit = []
    for g in range(2):
        gsl = slice(g * F, (g + 1) * F)
        m = {}
        for nm, wmat in (("q", qkv_w[gsl]),
                         ("k", qkv_w[E + g * F:E + (g + 1) * F]),
                         ("v", qkv_w[2 * E + g * F:2 * E + (g + 1) * F])):
            wT = np.ascontiguousarray(wmat.T) * WS  # [E, F] prescaled
            hi, res = _hi_res(wT)
            m[f"w{nm}"] = _pairs_w(hi.astype(np.float32),
                                   res.astype(np.float32)).astype(_ne4m3)
        wsplit.append(m)

    in_maps = []
    for c in range(N_CORES):
        b, g = c // 2, c % 2
        gsl = slice(g * F, (g + 1) * F)
        pwT = np.ascontiguousarray(proj_w[:, gsl].T)  # [F, E]
        m = {
            "x1": xsplit[b][0],
            "xr": xsplit[b][1],
            "pwT": np.ascontiguousarray(
                pwT.reshape(FB, 128, E).transpose(1, 0, 2)
            ).reshape(128, FB * E).astype(_nbf16),
            "bq": np.ascontiguousarray(qkv_b[gsl].reshape(FB, 128).T
                                       ).astype(np.float32),
            # const/2 is added pre-ReduceScatter by both pair cores;
            # col ob = const[ob block] / 2
            "constv": np.ascontiguousarray(
                const.reshape(OB, 128).T / 2.0).astype(np.float32),
            "maskp": maskp,
        }
        m.update(wsplit[g])
        in_maps.append(m)
    return in_maps


def assemble_output(results):
    y = np.empty((B, T, E), np.float32)
    for c in range(N_CORES):
        b, g = c // 2, c % 2
        # [512, T] bf16: row block ob (64 rows) = out-features
        # [128*ob + 64*g, 128*ob + 64*g + 64)
        o = np.asarray(results[c]["out"], dtype=np.float32)
        for ob in range(OB):
            lo = ob * 128 + 64 * g
            y[b][:, lo:lo + 64] = o[ob * 64:(ob + 1) * 64].T
    return y


def kernel(**inputs):
    nc = _get_nc()
    in_maps = make_in_maps(**inputs)
    res = run_bass_kernel_spmd(nc, in_maps, list(range(N_CORES)))
    return assemble_output(res.results)
